# revision 40
# baseline (speedup 1.0000x reference)
"""CSWin-style cross-attention block for Trainium2 (Bass/Tile), 8-core data-parallel.

v9 = v2 compute + stall-free PE queue (v5) + DMA-XBAR V^T (v8) + transposed-A
LePE (v9). PE-stream-bound ~410-430us with ACT exp (~266us busy) hidden.

Scheduling (v5; pure emission reordering, math identical to v2): exp drains a
score tile 2.4x slower than the PE refills it, so the v2 baseline lost ~90us
with scores(kc+2) blocking the in-order PE queue on exp(kc). Now:
  - branch A's LePE halves are emitted BEFORE its kc=2/3 score chunks, where
    the sct-ring wait would otherwise stall the queue;
  - branch A's AV+den partials (tile_position'd, so their PSUM accumulation
    groups tolerate interleaving) are woven between branch B's score chunks;
  - branch B's LePE runs after branch A's cat-add (earlier placements
    deadlock the lp ring against the DVE queue); qkv chunks fill the same
    slot in the B-window loop; expsb holds 16 tiles so both branches' exps
    stay live across the pipelined phases.
Tiling/engine placement:
  - host passes x pre-transposed [256, 4096]; QKV writes q/k/v directly into
    window-major layouts (A = 64x8 column strips, B = 8x64 row strips).
  - scores: S^T per (window, head, kchunk), 4 heads row-packed at array rows
    32h; exp on ACT (scale folded), bf16 out.
  - AV: att packed [128 = 4h x 32, 512]; denominator via all-ones lhsT,
    4-way col-tiled; normalize = recip / mul / add on DVE. (Fusing den into
    a 64-wide [vt|ones] lhsT is SLOWER on HW: only 2 array positions fit,
    and position reuse serializes weight loads - keep 8 narrow matmuls.)
  - V^T via DMA XBAR transposes straight into vt_sb (off the PE).
  - LePE 3x3 depthwise conv: 9 diagonal-weight matmuls over persistent
    zero-padded buffers; A windows are processed TRANSPOSED (8 rows x 66
    pitch like B) so each tap-half streams 264 cols instead of 320.
  - proj^T tail: out^T[oc, tok] = sum_b pw_b^T @ cat_b, bias on DVE.
"""
import os
import sys

sys.path.insert(0, "/opt/trn_rl_repo")
import numpy as np
import ml_dtypes

import concourse.bacc as bacc
import concourse.mybir as mybir
import concourse.tile as tile
from concourse.bass_utils import run_bass_kernel_spmd
from concourse.masks import make_identity

BF = mybir.dt.bfloat16
F32 = mybir.dt.float32
AF = mybir.ActivationFunctionType
ALU = mybir.AluOpType
SCALE = float(32.0 ** -0.5)

# tap order: (0,0) first so the start=True matmul covers the whole region
TAPS = [(0, 0)] + [(dr, dj) for dr in (-1, 0, 1) for dj in (-1, 0, 1) if (dr, dj) != (0, 0)]

# branch -> (combo, qhalf, kvhalf); combo A = 64x8 windows, B = 8x64
BRANCH = {0: ("A", 0, 0), 1: ("B", 1, 1), 2: ("A", 1, 0), 3: ("B", 0, 1)}

# padded flat window layouts for LePE: (rows, cols, row_pitch, region_base, total)
PAD = {"A": (8, 64, 66, 80, 688), "B": (8, 64, 66, 80, 688)}


def build(nc, debug=False, repeat=1, dyn_loop=0, with_cbias=True, probe=frozenset()):
    xT_d = nc.dram_tensor("xT", [256, 4096], BF, kind="ExternalInput").ap()
    qw = nc.dram_tensor("qw", [256, 768], BF, kind="ExternalInput").ap()
    pw = nc.dram_tensor("pw", [512, 256], BF, kind="ExternalInput").ap()
    dg = nc.dram_tensor("dg", [36, 128, 128], BF, kind="ExternalInput").ap()
    pbT = nc.dram_tensor("pbT", [128, 2], F32, kind="ExternalInput").ap()
    cb = nc.dram_tensor("cb", [128, 4], F32, kind="ExternalInput").ap()
    out_d = nc.dram_tensor("out", [256, 4096], F32, kind="ExternalOutput").ap()
    dbg = {}
    if debug:
        for name, shape in [("d_sc", [128, 2048]), ("d_exp", [128, 2048]),
                            ("d_av", [128, 512]), ("d_den", [128, 512]),
                            ("d_lep", [128, 688]), ("d_cat", [128, 512]),
                            ("d_vt", [128, 512]), ("d_q", [128, 512]),
                            ("d_k", [128, 512]), ("d_v", [128, 512])]:
            dbg[name] = nc.dram_tensor(name, shape, F32, kind="ExternalOutput").ap()

    with tile.TileContext(nc) as tc:
        with tc.sbuf_pool(name="persist", bufs=1) as ps_pool:
            # ---- constants / weights ----
            ident = ps_pool.tile([128, 128], BF, name="ident")
            make_identity(nc, ident)
            ones512 = ps_pool.tile([128, 512], BF, name="ones512")
            nc.vector.memset(ones512, 1.0)
            cbdiag = ps_pool.tile([128, 4 * 128], BF, name="cbdiag")

            qw_t = [ps_pool.tile([128, 768], BF, name=f"qw{i}") for i in range(2)]
            for i in range(2):
                nc.sync.dma_start(qw_t[i], qw[128 * i:128 * (i + 1), :])
            pw_t = [ps_pool.tile([128, 256], BF, name=f"pw{i}") for i in range(4)]
            for i in range(4):
                nc.sync.dma_start(pw_t[i], pw[128 * i:128 * (i + 1), :])
            diag_t = ps_pool.tile([128, 36 * 128], BF, name="diag_t")
            nc.sync.dma_start(diag_t.rearrange("p (t c) -> p t c", t=36),
                              dg.rearrange("t p c -> p t c"))
            pbT_t = ps_pool.tile([128, 2], F32, name="pbT_t")
            nc.sync.dma_start(pbT_t, pbT)
            cb_t = ps_pool.tile([128, 4], F32, name="cb_t")
            nc.sync.dma_start(cb_t, cb)
            for _b in range(4):
                nc.vector.tensor_scalar(cbdiag[:, 128 * _b:128 * (_b + 1)], ident,
                                        cb_t[:, _b:_b + 1], None, ALU.mult)

            # ---- persistent activations ----
            # window-major q/k/v: A layout col = 512w + 8r + j (w=A-window),
            # B layout = row-major tokens (window w = cols 512w..512w+512).
            xT = [ps_pool.tile([128, 4096], BF, name=f"xT{i}") for i in range(2)]
            qA = [ps_pool.tile([128, 4096], BF, name=f"qA{i}") for i in range(2)]
            qB = [ps_pool.tile([128, 4096], BF, name=f"qB{i}") for i in range(2)]
            kA0 = ps_pool.tile([128, 4096], BF, name="kA0")
            vA0 = ps_pool.tile([128, 4096], BF, name="vA0")
            kB1 = ps_pool.tile([128, 4096], BF, name="kB1")
            vB1 = ps_pool.tile([128, 4096], BF, name="vB1")
            cat_t = [ps_pool.tile([128, 4096], BF, name=f"cat{i}") for i in range(4)]
            # persistent zero-padded LePE buffers (pad cells stay zero; the
            # data region is overwritten per window). 2 per combo: ping-pong.
            vpadA = [ps_pool.tile([128, PAD["A"][4]], BF, name=f"vpadA{i}") for i in range(2)]
            vpadB = [ps_pool.tile([128, PAD["B"][4]], BF, name=f"vpadB{i}") for i in range(2)]
            for t in vpadA + vpadB:
                nc.vector.memset(t, 0.0)
            # ACT exp-table preload at t=0
            warm = ps_pool.tile([128, 1], F32, name="warm")
            nc.scalar.activation(warm, ones512[:, 0:1], AF.Exp, scale=1.0)

            # qkv dest map: m-block -> list of (dest tile, layout)
            #   m: 0=q0 1=q1 2=k0 3=k1 4=v0 5=v1
            qkv_dest = {
                0: [(qB[0], "B"), (qA[0], "A")],
                1: [(qB[1], "B"), (qA[1], "A")],
                2: [(kA0, "A")],
                3: [(kB1, "B")],
                4: [(vA0, "A")],
                5: [(vB1, "B")],
            }

            def _emit(_rep):
                with tc.tile_pool(name=f"scps{_rep}", bufs=2, space="PSUM") as scps, \
                     tc.tile_pool(name=f"avps{_rep}", bufs=1, space="PSUM") as avps, \
                     tc.tile_pool(name=f"dnps{_rep}", bufs=1, space="PSUM") as dnps, \
                     tc.tile_pool(name=f"auxps{_rep}", bufs=2, space="PSUM") as auxps, \
                     tc.sbuf_pool(name=f"expsb{_rep}", bufs=16) as expsb, \
                     tc.sbuf_pool(name=f"stg{_rep}", bufs=2) as stg, \
                     tc.sbuf_pool(name=f"outsb{_rep}", bufs=4) as outsb:

                    def qkv_chunk(n):
                        """token chunk n (512 tokens): DMA xT cols, 6 QKV matmuls,
                        scatter copies into layout tiles."""
                        for cc in range(2):
                            nc.sync.dma_start(xT[cc][:, 512 * n:512 * (n + 1)],
                                              xT_d[128 * cc:128 * (cc + 1),
                                                   512 * n:512 * (n + 1)])
                        for m in range(6):
                            qp = auxps.tile([128, 512], F32, tag="aux", name="qp")
                            for cc in range(2):
                                nc.tensor.matmul(qp, qw_t[cc][:, 128 * m:128 * (m + 1)],
                                                 xT[cc][:, 512 * n:512 * (n + 1)],
                                                 start=(cc == 0), stop=(cc == 1),
                                                 skip_group_check=True)
                            for dst, layout in qkv_dest[m]:
                                if layout == "B":
                                    nc.vector.tensor_copy(dst[:, 512 * n:512 * (n + 1)], qp)
                                else:
                                    # A scatter: dst col = 512w + 8r + j with
                                    # r = 8n + r8; src col = 64r8 + 8w + j
                                    dv = dst.rearrange("c (w r j) -> c w r j",
                                                       w=8, r=64, j=8)[:, :, 8 * n:8 * (n + 1), :]
                                    sv = qp.rearrange("c (r w j) -> c w r j",
                                                      r=8, w=8, j=8)
                                    nc.vector.tensor_copy(dv, sv)

                    def window_pair(combo, w, filler=None):
                        R, J, T, RB, TOT = PAD[combo]
                        if combo == "A":
                            branches, kwin_t, vwin_t, vpad_t = (0, 2), kA0, vA0, vpadA[w % 2]
                            qsrc = qA
                        else:
                            branches, kwin_t, vwin_t, vpad_t = (1, 3), kB1, vB1, vpadB[w % 2]
                            qsrc = qB
                        kwin = kwin_t[:, 512 * w:512 * (w + 1)]
                        vwin = vwin_t[:, 512 * w:512 * (w + 1)]
                        is_dbg_w = debug and combo == "A" and w == 0

                        # ---- zero-padded v window for LePE (pad stays 0).
                        # A windows are written TRANSPOSED (row = image col j,
                        # 64-wide rows like B) so the padded stream is 264
                        # cols/half instead of 320. ----
                        dstv = vpad_t[:, RB:RB + R * T].rearrange(
                            "c (r t) -> c r t", t=T)[:, :, 1:1 + J]
                        if combo == "A":
                            nc.vector.tensor_copy(
                                dstv, vwin.rearrange("c (r j) -> c j r", j=R))
                        else:
                            nc.vector.tensor_copy(
                                dstv, vwin.rearrange("c (r j) -> c r j", j=J))

                        # ---- V^T via DMA XBAR transposes (off the PE;
                        # contiguous [128,128] blocks land directly in vt_sb) ----
                        vt_sb = stg.tile([128, 512], BF, tag="vt", name="vt_sb")
                        for kc in range(4):
                            nc.sync.dma_start_transpose(
                                vt_sb[:, 128 * kc:128 * (kc + 1)],
                                vwin[:, 128 * kc:128 * (kc + 1)])
                        if is_dbg_w:
                            vt32 = stg.tile([128, 512], F32, tag="dbgvt", name="vt32")
                            nc.vector.tensor_copy(vt32, vt_sb)
                            nc.sync.dma_start(dbg["d_vt"], vt32)

                        # stall-free pipelined emission: LePE halves sit
                        # BEFORE the score chunks that would stall on the sct
                        # ring (exp drains sct 2.4x slower than scores fill
                        # it), and branch A's AV+den partials interleave with
                        # branch B's score chunks (legal across kc: these
                        # matmuls carry tile_position).
                        st = {br: {"exp": {}, "lps": [None, None],
                                   "att": None, "den": None} for br in branches}

                        def lepe_half(br, half):
                            hr = R // 2
                            hspan = hr * T
                            lp = auxps.tile([128, hspan], F32, tag="aux", name="lp")
                            st[br]["lps"][half] = lp
                            base = RB + half * hspan
                            for t, (dr, dj) in enumerate(TAPS):
                                delta = (T * dj + dr) if combo == "A" else (T * dr + dj)
                                dmat = diag_t[:, (br * 9 + t) * 128:(br * 9 + t + 1) * 128]
                                nc.tensor.matmul(
                                    lp,
                                    dmat,
                                    vpad_t[:, base + delta:base + delta + hspan],
                                    start=(t == 0),
                                    stop=(not with_cbias and t == 8),
                                    skip_group_check=True)
                            if with_cbias:
                                nc.tensor.matmul(
                                    lp,
                                    cbdiag[:, 128 * br:128 * (br + 1)],
                                    ones512[:, 0:hspan],
                                    start=False, stop=True, skip_group_check=True)

                        def scores_kc(br, kc):
                            qfull = qsrc[BRANCH[br][1]][:, 512 * w:512 * (w + 1)]
                            sct = [scps.tile([128, 1024], F32, tag="sc", name="sct")
                                   for _ in range(2)]
                            for h in range(4):
                                nc.tensor.matmul(
                                    sct[h // 2][:, 512 * (h % 2):512 * (h % 2) + 512],
                                    kwin[32 * h:32 * (h + 1), 128 * kc:128 * (kc + 1)],
                                    qfull[32 * h:32 * (h + 1), :],
                                    start=True, stop=True,
                                    tile_position=(32 * h, 0))
                            for p in range(2):
                                e = expsb.tile([128, 1024], BF, tag="exp", name="exp")
                                if "act_lite" in probe:
                                    nc.scalar.activation(e[:, :128], sct[p][:, :128],
                                                         AF.Exp, scale=SCALE)
                                else:
                                    nc.scalar.activation(e, sct[p], AF.Exp, scale=SCALE)
                                st[br]["exp"][(p, kc)] = e

                        def avden_kc(br, kc):
                            s = st[br]
                            if s["att"] is None:
                                s["att"] = avps.tile([128, 512], F32, tag="av", name="att")
                                s["den"] = dnps.tile([128, 512], F32, tag="dn", name="den")
                            att, den, exp_tiles = s["att"], s["den"], s["exp"]
                            for h in range(4):
                                nc.tensor.matmul(
                                    att[32 * h:32 * (h + 1), :],
                                    vt_sb[:, 128 * kc + 32 * h:128 * kc + 32 * (h + 1)],
                                    exp_tiles[(h // 2, kc)][:, 512 * (h % 2):512 * (h % 2) + 512],
                                    start=(kc == 0), stop=(kc == 3),
                                    tile_position=(0, 32 * h), skip_group_check=True)
                            for h in range(4):
                                if "den_lite" in probe and kc > 0:
                                    continue
                                nc.tensor.matmul(
                                    den[32 * h:32 * (h + 1), :],
                                    ones512[:, 0:32],
                                    exp_tiles[(h // 2, kc)][:, 512 * (h % 2):512 * (h % 2) + 512],
                                    start=(kc == 0),
                                    stop=(kc == 3 or "den_lite" in probe),
                                    tile_position=(0, 32 * h), skip_group_check=True)

                        def normalize_cat(br):
                            s = st[br]
                            hr = R // 2
                            rd = stg.tile([128, 512], F32, tag="recip", name="rd")
                            nc.vector.reciprocal_approx_fast(rd, s["den"])
                            t_sb = stg.tile([128, 512], F32, tag="tsb", name="t_sb")
                            nc.vector.tensor_mul(t_sb, s["att"], rd)
                            if combo == "A":
                                # halves split the j axis (4 j-rows each); all
                                # operands viewed as [c, 64 r, 4 j]
                                catw = cat_t[br].rearrange(
                                    "c (r w j) -> c w r j", r=64, w=8, j=8)[:, w]
                                for half in range(2):
                                    lpv = s["lps"][half].rearrange(
                                        "c (j t) -> c j t", t=T)[:, :, 1:1 + J]
                                    lpv = lpv.rearrange("c j r -> c r j")
                                    t3 = t_sb.rearrange(
                                        "c (r j) -> c r j", j=8)[:, :, 4 * half:4 * half + 4]
                                    nc.vector.tensor_add(
                                        catw[:, :, 4 * half:4 * half + 4], lpv, t3)
                            else:
                                catw = cat_t[br][:, 512 * w:512 * (w + 1)].rearrange(
                                    "c (r j) -> c r j", j=J)
                                for half in range(2):
                                    lpv = s["lps"][half].rearrange(
                                        "c (r t) -> c r t", t=T)[:, :, 1:1 + J]
                                    t3 = t_sb[:, hr * J * half:hr * J * (half + 1)].rearrange(
                                        "c (a b) -> c a b", a=hr, b=J)
                                    nc.vector.tensor_add(catw[:, hr * half:hr * (half + 1), :],
                                                         lpv, t3)

                        brA, brB = branches
                        for kc in range(4):
                            if kc >= 2:
                                lepe_half(brA, kc - 2)
                            scores_kc(brA, kc)
                        for kc in range(4):
                            avden_kc(brA, kc)
                            scores_kc(brB, kc)
                        normalize_cat(brA)
                        if filler is not None:
                            filler()
                        lepe_half(brB, 0)
                        lepe_half(brB, 1)
                        for kc in range(4):
                            avden_kc(brB, kc)
                        normalize_cat(brB)

                    # ============ emission: QKV pipelined with B, then A ============
                    qkv_chunk(0)
                    qkv_chunk(1)
                    for w in range(8):
                        window_pair("B", w,
                                    filler=(lambda n=w + 2: qkv_chunk(n))
                                    if w + 2 < 8 else None)
                    for w in range(8):
                        window_pair("A", w)

                    # ============ proj^T tail ============
                    pools = [avps, dnps, auxps]
                    tags = ["av", "dn", "aux"]
                    for u, (n, oh) in enumerate([(n, oh) for n in range(8) for oh in range(2)]):
                        pool = pools[u % 3]
                        pp = pool.tile([128, 512], F32, tag=tags[u % 3], name="pp")
                        for b2 in range(4):
                            nc.tensor.matmul(pp, pw_t[b2][:, 128 * oh:128 * (oh + 1)],
                                             cat_t[b2][:, 512 * n:512 * (n + 1)],
                                             start=(b2 == 0), stop=(b2 == 3),
                                             skip_group_check=True)
                        osb = outsb.tile([128, 512], F32, tag="out", name="osb")
                        nc.vector.tensor_scalar(osb, pp, pbT_t[:, oh:oh + 1], None, ALU.add)
                        nc.sync.dma_start(out_d[128 * oh:128 * (oh + 1),
                                                512 * n:512 * (n + 1)], osb)

            if dyn_loop:
                with tc.For_i(0, dyn_loop, 1):
                    _emit(0)
            else:
                for _rep in range(repeat):
                    _emit(_rep)

    return nc


_CACHE = {}


def _get_nc(debug=False, repeat=1, dyn_loop=0, with_cbias=True, probe=frozenset()):
    key = (bool(debug), repeat, dyn_loop, with_cbias, probe)
    if key not in _CACHE:
        nc = bacc.Bacc("TRN2", target_bir_lowering=False, debug=False)
        build(nc, debug=debug, repeat=repeat, dyn_loop=dyn_loop, with_cbias=with_cbias,
              probe=probe)
        nc.compile()
        _CACHE[key] = nc
    return _CACHE[key]


def prep_inputs(x, qkv_w, proj_w, proj_b, conv_ws, conv_bs):
    x = np.asarray(x)
    B = x.shape[0]
    qwb = np.asarray(qkv_w).astype(ml_dtypes.bfloat16)
    pwb = np.asarray(proj_w).astype(ml_dtypes.bfloat16)
    w9 = np.asarray(conv_ws).reshape(4, 128, 9).astype(np.float32)
    dgn = np.zeros((36, 128, 128), np.float32)
    idx = np.arange(128)
    for br in range(4):
        for t, (dr, dj) in enumerate(TAPS):
            dgn[br * 9 + t, idx, idx] = w9[br, :, (dr + 1) * 3 + (dj + 1)]
    dgn = dgn.astype(ml_dtypes.bfloat16)
    pbTn = np.ascontiguousarray(
        np.asarray(proj_b, np.float32).reshape(2, 128).T)
    cbt = np.ascontiguousarray(np.asarray(conv_bs, np.float32).T)
    shared = {"qw": qwb, "pw": pwb, "dg": dgn, "pbT": pbTn, "cb": cbt}
    return [dict(shared,
                 xT=np.ascontiguousarray(x[b].T.astype(ml_dtypes.bfloat16)))
            for b in range(B)]


def kernel(x, qkv_w, proj_w, proj_b, conv_ws, conv_bs, _debug=False, _trace=False):
    wcb = bool(np.any(np.asarray(conv_bs)))
    nc = _get_nc(debug=_debug, with_cbias=wcb)
    in_maps = prep_inputs(x, qkv_w, proj_w, proj_b, conv_ws, conv_bs)
    res = run_bass_kernel_spmd(nc, in_maps, core_ids=list(range(len(in_maps))),
                               trace=_trace)
    out = np.stack([np.ascontiguousarray(r["out"].T) for r in res.results]
                   ).astype(np.float32)
    if _debug or _trace:
        kernel.last_results = res
    return out

